# revision 5
# baseline (speedup 1.0000x reference)
"""Trainium2 Bass kernel for nn_LEAP_74371653697613 (GRU decoder w/ additive attention).

v2: Picard-sweep formulation of the recurrence.

Structure exploited:
  - softmax(ctx_score + h.w_h) == softmax(ctx_score): attention weights constant
    across steps -> context c computed once.
  - gi_t = W_ih @ [c; x_t] + b_ih precomputed batched (teacher forcing).
  - The h-recurrence h_t = (1-z_t) n_t + z_t h_{t-1} is solved by fixed-point
    iteration over the WHOLE sequence: each sweep computes gh = W_hh @ H_prev as
    ONE batched [65,3072] matmul (W_hh streamed once per sweep instead of once
    per step), gates batched, then the exact sequential structure is restored by
    the DVE prefix-scan  state = (z_t * state) - (z_t - 1)*n_t  per hidden chunk.
    Contraction ~0.3x/sweep; KS sweeps reach the bf16 noise floor (~2.6e-3 rel).
  - Sweep 1 has H_prev === h0, so its gh is a single col-tiled matvec W_hh @ h0
    broadcast across t (cheap) instead of the batched matmul.
  - logits = relu(H) @ out_w^T batched at the end, vocab-sharded 8 ways
    (each core owns a 4096-row slice of out_w); out_w prefetched during sweeps.

outer_reps repeats phases 2-5 inside one NEFF for slope-based timing; the
shipped kernel uses outer_reps=1.
"""
import os
import sys
import numpy as np

for _p in ("/opt/trn_rl_repo", "/root/.axon_site/_ro/trn_rl_repo"):
    if os.path.isdir(_p) and _p not in sys.path:
        sys.path.insert(0, _p)

import concourse.bass as bass
import concourse.bacc as bacc
import concourse.tile as tile
import concourse.mybir as mybir
from concourse.bass_utils import run_bass_kernel_spmd
from concourse.masks import make_identity

F32 = mybir.dt.float32
BF16 = mybir.dt.bfloat16
AF = mybir.ActivationFunctionType
ALU = mybir.AluOpType
NP_BF16 = mybir.dt.np(BF16)

E = 1024          # emb dim
EC = 8            # E / 128 chunks
T = 65            # decode steps (1 SOS + 64)
L = 320           # context rows (128 + 64 + 128)
V0 = 32000
V = V0 + 2        # 32002
NCORES = 8
VP = 4096         # per-core padded vocab slice (8 * 4096 = 32768 >= 32002)
G = 4             # gate regions (each 256 hidden units x 3 gates)
RG = 768          # region width (3 gates x 256)
CW = G * RG       # per-K-chunk weight width (3072)
KS = 6            # Picard sweeps (incl. the specialized first sweep)
TP = 66           # padded t-stride in hprevT: slot 0 = h0, slots 1..65 = scan out

_CACHE = {}


def _arrange_w(w):
    """[3072, 1024] -> [128, 8*4*768]: out[p, ((c*4)+j)*768 + g*256+mm]
    = w[g*1024 + j*256 + mm, c*128 + p]."""
    x = w.reshape(3, 4, 256, EC, 128)            # g, j, mm, c, p
    x = np.transpose(x, (4, 3, 1, 0, 2))         # p, c, j, g, mm
    return np.ascontiguousarray(x).reshape(128, EC * CW)


def _bias_tall(b_rzn):
    x = b_rzn.reshape(3, 4, 256)                 # g, j, mm
    x = np.transpose(x, (1, 0, 2)).reshape(4, RG)  # j, (g mm)
    out = np.zeros((128, RG), np.float32)
    out[::32, :] = x
    return out


def build_program(ksweeps=KS, do_final=True, outer_reps=1, with_bias=True):
    nc = bacc.Bacc("TRN2", target_bir_lowering=False, debug=False, num_devices=NCORES)

    ctx_d = nc.dram_tensor("ctx", [L, E], F32, kind="ExternalInput").ap()
    decx_d = nc.dram_tensor("decx", [T, E], F32, kind="ExternalInput").ap()
    we_d = nc.dram_tensor("we", [1, E], F32, kind="ExternalInput").ap()
    whh_d = nc.dram_tensor("whh", [128, EC * CW], BF16, kind="ExternalInput").ap()
    wc_d = nc.dram_tensor("wc", [128, EC * CW], BF16, kind="ExternalInput").ap()
    wx_d = nc.dram_tensor("wx", [128, EC * CW], BF16, kind="ExternalInput").ap()
    bias_d = nc.dram_tensor("bias", [128, RG], F32, kind="ExternalInput").ap()
    owt_d = nc.dram_tensor("owt", [128, EC * VP], BF16, kind="ExternalInput").ap()
    outb_d = nc.dram_tensor("outb", [1, VP], F32, kind="ExternalInput").ap()
    out_d = nc.dram_tensor("out", [T, VP], BF16, kind="ExternalOutput").ap()

    with tile.TileContext(nc) as tc:
        with tc.tile_pool(name="persist", bufs=1) as pp:
            # ---------- persistent constants ----------
            whh = pp.tile([128, EC * CW], BF16)
            for c in range(EC):
                nc.sync.dma_start(whh[:, c * CW:(c + 1) * CW],
                                  whh_d[:, c * CW:(c + 1) * CW])

            ident = pp.tile([128, 128], F32)
            make_identity(nc, ident[:])
            ident_bf = pp.tile([128, 128], BF16)
            nc.vector.tensor_copy(ident_bf[:], ident[:])

            ones_tall = pp.tile([128, T], F32)
            nc.gpsimd.memset(ones_tall[:], 1.0)

            bias_tall = pp.tile([128, RG], F32)
            nc.sync.dma_start(bias_tall[:], bias_d[:])

            girz = pp.tile([T, G, 512], BF16)      # gi rz-part, partition = t
            gin65 = pp.tile([T, 1024], F32)        # gi n-part, partition = t
            gic_tall = pp.tile([128, RG], F32)     # const part of gi, rows at 32j
            h_stat = pp.tile([128, EC], F32)       # h0 chunks (scan initial)
            h0bf = pp.tile([128, EC], BF16)
            gh0 = pp.tile([128, RG], F32)          # W_hh @ h0, region rows at 32j
            hprevT = pp.tile([128, EC * TP], BF16)  # slot (c,0)=h0; (c,1..65)=h_1..h_65
            htf = pp.tile([128, EC * T], BF16)      # relu(h_1..h_65)
            cT_bf = pp.tile([128, EC], BF16)
            dxT_bf = pp.tile([128, EC, T], BF16)

            # h0 = dec_emb[SOS] = decx row 0 in stationary layout
            nc.sync.dma_start(h_stat[:], decx_d[0:1, :].rearrange("o (c p) -> (o p) c", p=128))
            nc.vector.tensor_copy(h0bf[:], h_stat[:])

            whhv = whh[:].rearrange("p (c j m) -> p c j m", c=EC, j=G)
            hprevT_v = hprevT[:].rearrange("p (c t) -> p c t", c=EC)

            # ---------- phase 1: attention (constant across steps) ----------
            with tc.tile_pool(name="ph1", bufs=1) as p1, \
                 tc.tile_pool(name="ph1ps", bufs=1, space="PSUM") as p1ps:
                ones_col = p1.tile([128, 1], F32)
                nc.gpsimd.memset(ones_col[:], 1.0)
                ones_row = p1.tile([1, 128], F32)
                nc.gpsimd.memset(ones_row[:], 1.0)
                we_sb = p1.tile([1, E], F32)
                nc.sync.dma_start(we_sb[:], we_d[:])
                rows3 = (128, 128, 64)
                ctx_sb = []
                for i, rows in enumerate(rows3):
                    t_ = p1.tile([128, E], F32, tag=f"ctx{i}")
                    nc.sync.dma_start(t_[:rows, :], ctx_d[128 * i:128 * i + rows, :])
                    ctx_sb.append(t_)
                decx_sb = p1.tile([T, E], F32)
                nc.sync.dma_start(decx_sb[:], decx_d[:])

                werep_ps = p1ps.tile([128, E], F32, space="PSUM")
                for half in range(2):
                    nc.tensor.matmul(werep_ps[:, 512 * half:512 * (half + 1)],
                                     lhsT=ones_row[:1, :],
                                     rhs=we_sb[:1, 512 * half:512 * (half + 1)],
                                     start=True, stop=True)
                werep = p1.tile([128, E], F32)
                nc.vector.tensor_copy(werep[:], werep_ps[:])

                scratch = p1.tile([128, E], F32)
                escore = [p1.tile([128, 1], F32, tag=f"esc{i}", name=f"esc{i}")
                          for i in range(3)]
                for i, rows in enumerate(rows3):
                    sc = p1.tile([128, 1], F32, tag=f"sc{i}")
                    nc.vector.tensor_tensor(out=scratch[:rows, :],
                                            in0=ctx_sb[i][:rows, :],
                                            in1=werep[:rows, :], op=ALU.mult)
                    nc.vector.tensor_reduce(out=sc[:rows, :], in_=scratch[:rows, :],
                                            axis=mybir.AxisListType.X, op=ALU.add)
                    nc.scalar.activation(escore[i][:rows, :], sc[:rows, :], AF.Exp)
                ssum_ps = p1ps.tile([1, 1], F32, space="PSUM")
                for i, rows in enumerate(rows3):
                    nc.tensor.matmul(ssum_ps[:1, :1], lhsT=escore[i][:rows, :1],
                                     rhs=ones_col[:rows, :1],
                                     start=(i == 0), stop=(i == 2))
                rsum = p1.tile([1, 1], F32)
                nc.vector.reciprocal(rsum[:], ssum_ps[:1, :1])

                cun_ps = p1ps.tile([1, E], F32, space="PSUM")
                for half in range(2):
                    for i, rows in enumerate(rows3):
                        nc.tensor.matmul(cun_ps[:1, 512 * half:512 * (half + 1)],
                                         lhsT=escore[i][:rows, :1],
                                         rhs=ctx_sb[i][:rows, 512 * half:512 * (half + 1)],
                                         start=(i == 0), stop=(i == 2))
                c_sb = p1.tile([1, E], F32)
                nc.vector.tensor_scalar_mul(c_sb[:], cun_ps[:1, :], rsum[:1, :1])

                cT_ps = p1ps.tile([128, EC], F32, space="PSUM")
                for k in range(EC):
                    nc.tensor.transpose(out=cT_ps[:, k:k + 1],
                                        in_=c_sb[:1, 128 * k:128 * (k + 1)],
                                        identity=ident[:1, :1])
                nc.vector.tensor_copy(cT_bf[:], cT_ps[:])

                dxT_ps = p1ps.tile([128, T], F32, space="PSUM")
                for k in range(EC):
                    nc.tensor.transpose(out=dxT_ps[:, :],
                                        in_=decx_sb[:T, 128 * k:128 * (k + 1)],
                                        identity=ident[:T, :T])
                    nc.vector.tensor_copy(dxT_bf[:, k, :], dxT_ps[:, :])

            for rep in range(outer_reps):
                # ---------- phase 2: gic = W_ih[:, :E] @ c + biases ----------
                with tc.tile_pool(name=f"pwc{rep}", bufs=1) as pwc, \
                     tc.tile_pool(name=f"pwcps{rep}", bufs=1, space="PSUM") as pwcps:
                    wc_sb = pwc.tile([128, EC * CW], BF16)
                    for c in range(EC):
                        nc.sync.dma_start(wc_sb[:, c * CW:(c + 1) * CW],
                                          wc_d[:, c * CW:(c + 1) * CW])
                    wcv = wc_sb[:].rearrange("p (c j m) -> p c j m", c=EC, j=G)
                    gic_ps = pwcps.tile([128, 1024], F32, space="PSUM")
                    for c in range(EC):
                        for j in range(G):
                            nc.tensor.matmul(gic_ps[32 * j:32 * j + 1, 0:512],
                                             lhsT=cT_bf[:, c:c + 1],
                                             rhs=wcv[:, c, j, 0:512],
                                             start=(c == 0), stop=False,
                                             tile_position=(0, 32 * j))
                            nc.tensor.matmul(gic_ps[32 * j:32 * j + 1, 512:768],
                                             lhsT=cT_bf[:, c:c + 1],
                                             rhs=wcv[:, c, j, 512:768],
                                             start=(c == 0), stop=False,
                                             tile_position=(0, 32 * j))
                    for j in range(G):
                        nc.tensor.matmul(gic_ps[32 * j:32 * j + 1, 0:512],
                                         lhsT=ones_tall[32 * j:32 * j + 1, 0:1],
                                         rhs=bias_tall[32 * j:32 * j + 1, 0:512],
                                         start=False, stop=True,
                                         tile_position=(32 * j, 32 * j))
                        nc.tensor.matmul(gic_ps[32 * j:32 * j + 1, 512:768],
                                         lhsT=ones_tall[32 * j:32 * j + 1, 0:1],
                                         rhs=bias_tall[32 * j:32 * j + 1, 512:768],
                                         start=False, stop=True,
                                         tile_position=(32 * j, 32 * j))
                    for j in range(G):
                        if j % 2 == 0:
                            nc.scalar.copy(gic_tall[32 * j:32 * j + 1, :],
                                           gic_ps[32 * j:32 * j + 1, 0:RG])
                        else:
                            nc.vector.tensor_copy(gic_tall[32 * j:32 * j + 1, :],
                                                  gic_ps[32 * j:32 * j + 1, 0:RG])

                # ---------- phase 3: gi[t] = gic + W_ih[:, E:] @ x_t (batched) ----------
                with tc.tile_pool(name=f"pwx{rep}", bufs=1) as pwx, \
                     tc.tile_pool(name=f"pwxps{rep}", bufs=1, space="PSUM") as pwxps:
                    wx_sb = pwx.tile([128, EC * CW], BF16)
                    for c in range(EC):
                        nc.sync.dma_start(wx_sb[:, c * CW:(c + 1) * CW],
                                          wx_d[:, c * CW:(c + 1) * CW])
                    wxv = wx_sb[:].rearrange("p (c j m) -> p c j m", c=EC, j=G)
                    rzts = [pwxps.tile([T, 512], F32, space="PSUM", tag=f"grz{j}",
                                       name=f"grz{j}") for j in range(G)]
                    npts = [pwxps.tile([T, 256], F32, space="PSUM", tag=f"gn{j}",
                                       name=f"gn{j}") for j in range(G)]
                    for c in range(EC):
                        for j in range(G):
                            nc.tensor.matmul(rzts[j][:T, :], lhsT=dxT_bf[:, c, :],
                                             rhs=wxv[:, c, j, 0:512],
                                             start=(c == 0), stop=False)
                            nc.tensor.matmul(npts[j][:T, :], lhsT=dxT_bf[:, c, :],
                                             rhs=wxv[:, c, j, 512:768],
                                             start=(c == 0), stop=False)
                    for j in range(G):
                        nc.tensor.matmul(rzts[j][:T, :],
                                         lhsT=ones_tall[32 * j:32 * j + 1, :T],
                                         rhs=gic_tall[32 * j:32 * j + 1, 0:512],
                                         start=False, stop=True,
                                         tile_position=(32 * j, 0))
                        nc.vector.tensor_copy(girz[:, j, :], rzts[j][:T, :])
                        nc.tensor.matmul(npts[j][:T, :],
                                         lhsT=ones_tall[32 * j:32 * j + 1, :T],
                                         rhs=gic_tall[32 * j:32 * j + 1, 512:768],
                                         start=False, stop=True,
                                         tile_position=(32 * j, 0))
                        nc.vector.tensor_copy(gin65[:, 256 * j:256 * (j + 1)],
                                              npts[j][:T, :])

                # prefetch final-phase weights (after whh/wc/wx in program order)
                if rep == 0:
                    owt_sb = pp.tile([128, EC * VP], BF16)
                    outb_sb = pp.tile([1, VP], F32)
                    if do_final:
                        nc.sync.dma_start(owt_sb[:], owt_d[:])
                        nc.sync.dma_start(outb_sb[:], outb_d[:])

                # ---------- phase 4: Picard sweeps ----------
                # hprevT[:, c, 0] = h0 (sweeps 2+ read it; cols 1: from scans)
                nc.vector.tensor_copy(hprevT_v[:, :, 0:1], h_stat[:].unsqueeze(2))

                with tc.tile_pool(name=f"sw{rep}", bufs=4) as psw, \
                     tc.tile_pool(name=f"swg{rep}", bufs=2) as psg2, \
                     tc.tile_pool(name=f"swps{rep}", bufs=2, space="PSUM") as pps1, \
                     tc.tile_pool(name=f"swpsT{rep}", bufs=1, space="PSUM") as pps2:
                    # sweep-1 prologue: gh0 = W_hh @ h0 (col-tiled matvec, M=1).
                    # psg0 borrows the zT buffer (tag reuse; lifetimes disjoint).
                    psg0 = pps2.tile([128, 1024], F32, space="PSUM", tag="zT")
                    for c in range(EC):
                        for j in range(G):
                            nc.tensor.matmul(psg0[32 * j:32 * j + 1, 0:512],
                                             lhsT=h0bf[:, c:c + 1],
                                             rhs=whhv[:, c, j, 0:512],
                                             start=(c == 0), stop=(c == EC - 1),
                                             tile_position=(0, 32 * j))
                            nc.tensor.matmul(psg0[32 * j:32 * j + 1, 512:768],
                                             lhsT=h0bf[:, c:c + 1],
                                             rhs=whhv[:, c, j, 512:768],
                                             start=(c == 0), stop=(c == EC - 1),
                                             tile_position=(0, 32 * j))
                    for j in range(G):
                        if j % 2 == 0:
                            nc.scalar.copy(gh0[32 * j:32 * j + 1, :],
                                           psg0[32 * j:32 * j + 1, 0:RG])
                        else:
                            nc.vector.tensor_copy(gh0[32 * j:32 * j + 1, :],
                                                  psg0[32 * j:32 * j + 1, 0:RG])

                    for k in range(ksweeps):
                        first = (k == 0)
                        sgs, npres = [], []
                        # rz pass: 4 regions, gi fold + 8 K-chunks each
                        for j in range(G):
                            rz = pps1.tile([T, 512], F32, space="PSUM", tag="rz")
                            nc.tensor.matmul(rz[:T, :], lhsT=ident_bf[:T, :T],
                                             rhs=girz[:, j, :], start=True, stop=False)
                            if first:
                                nc.tensor.matmul(rz[:T, :],
                                                 lhsT=ones_tall[32 * j:32 * j + 1, :T],
                                                 rhs=gh0[32 * j:32 * j + 1, 0:512],
                                                 start=False, stop=True,
                                                 tile_position=(32 * j, 0))
                            else:
                                for c in range(EC):
                                    nc.tensor.matmul(rz[:T, :],
                                                     lhsT=hprevT[:, c * TP:c * TP + T],
                                                     rhs=whhv[:, c, j, 0:512],
                                                     start=False, stop=(c == EC - 1))
                            sg = psw.tile([T, 512], F32, tag="sg")
                            nc.scalar.activation(sg[:], rz[:T, :], AF.Sigmoid)
                            sgs.append(sg)
                        # n pass
                        for j in range(G):
                            nps_j = pps1.tile([T, 256], F32, space="PSUM", tag="n")
                            if first:
                                nc.tensor.matmul(nps_j[:T, :],
                                                 lhsT=ones_tall[32 * j:32 * j + 1, :T],
                                                 rhs=gh0[32 * j:32 * j + 1, 512:768],
                                                 start=True, stop=True,
                                                 tile_position=(32 * j, 0))
                            else:
                                for c in range(EC):
                                    nc.tensor.matmul(nps_j[:T, :],
                                                     lhsT=hprevT[:, c * TP:c * TP + T],
                                                     rhs=whhv[:, c, j, 512:768],
                                                     start=(c == 0), stop=(c == EC - 1))
                            t1 = psg2.tile([T, 256], F32, tag="t1")
                            nc.vector.tensor_tensor(out=t1[:], in0=sgs[j][:, 0:256],
                                                    in1=nps_j[:T, :], op=ALU.mult)
                            npre = psw.tile([T, 256], F32, tag="npre")
                            nc.vector.tensor_tensor(out=npre[:], in0=t1[:],
                                                    in1=gin65[:, 256 * j:256 * (j + 1)],
                                                    op=ALU.add)
                            npres.append(npre)
                        # transposes into [128, (half, cc, t)] PSUM; chunk cc at
                        # column 512*(cc//4) + 65*(cc%4)
                        zT = pps2.tile([128, 1024], F32, space="PSUM", tag="zT")
                        npT = pps2.tile([128, 1024], F32, space="PSUM", tag="npT")
                        for cc in range(EC):
                            j, k2 = cc // 2, cc % 2
                            col = 512 * (cc // 4) + T * (cc % 4)
                            nc.tensor.transpose(out=zT[:, col:col + T],
                                                in_=sgs[j][:T, 256 + 128 * k2:256 + 128 * (k2 + 1)],
                                                identity=ident[:T, :T])
                            nc.tensor.transpose(out=npT[:, col:col + T],
                                                in_=npres[j][:T, 128 * k2:128 * (k2 + 1)],
                                                identity=ident[:T, :T])
                        nT = psg2.tile([128, 1024], F32, tag="nT")
                        wsb = psg2.tile([128, 1024], F32, tag="wsb")
                        for h in range(2):
                            s = slice(512 * h, 512 * h + 4 * T)
                            nc.scalar.activation(nT[:, s], npT[:, s], AF.Tanh)
                            # (z - 1) * n
                            nc.vector.scalar_tensor_tensor(out=wsb[:, s], in0=zT[:, s],
                                                           scalar=1.0, in1=nT[:, s],
                                                           op0=ALU.subtract, op1=ALU.mult)
                        for cc in range(EC):
                            col = 512 * (cc // 4) + T * (cc % 4)
                            # state = z*state - (z-1)*n; writes h_1..h_65 into
                            # slots (cc, 1..65); next sweep's lhsT reads (cc, 0..64)
                            nc.vector.tensor_tensor_scan(
                                out=hprevT[:, cc * TP + 1:cc * TP + 1 + T],
                                data0=zT[:, col:col + T], data1=wsb[:, col:col + T],
                                initial=h_stat[:, cc:cc + 1],
                                op0=ALU.mult, op1=ALU.subtract)

                # ---------- phase 5: logits = relu(H) @ out_w^T + out_b ----------
                nc.scalar.activation(htf[:].rearrange("p (c t) -> p c t", c=EC),
                                     hprevT_v[:, :, 1:TP], AF.Relu)
                owtv = owt_sb[:].rearrange("p (c v) -> p c v", c=EC)
                htv = htf[:].rearrange("p (c t) -> p c t", c=EC)
                if not do_final and rep == 0:
                    nc.sync.dma_start(out_d[0:T, 0:T], htf[:T, 0:T])
                with tc.tile_pool(name=f"fin{rep}", bufs=2) as pf, \
                     tc.tile_pool(name=f"finps{rep}", bufs=2, space="PSUM") as pfps:
                    for vb in range(VP // 512 if do_final else 0):
                        ops = pfps.tile([T, 512], F32, space="PSUM", tag="ops")
                        for c in range(EC):
                            nc.tensor.matmul(ops[:T, :], lhsT=htv[:, c, :],
                                             rhs=owtv[:, c, 512 * vb:512 * (vb + 1)],
                                             start=(c == 0),
                                             stop=(not with_bias and c == EC - 1))
                        if with_bias:
                            nc.tensor.matmul(ops[:T, :], lhsT=ones_tall[:1, :T],
                                             rhs=outb_sb[:1, 512 * vb:512 * (vb + 1)],
                                             start=False, stop=True)
                        osb = pf.tile([T, 512], BF16, tag="osb")
                        if vb % 2 == 0:
                            nc.vector.tensor_copy(osb[:], ops[:T, :])
                        else:
                            nc.scalar.copy(osb[:], ops[:T, :])
                        nc.sync.dma_start(out_d[:, 512 * vb:512 * (vb + 1)], osb[:])

    nc.compile()
    return nc


def _prep_inputs(inp):
    idx_enc = np.concatenate([inp["input_diagnosis"], inp["input_procedure"],
                              inp["input_medicine"]]).astype(np.int64)
    tokens = np.concatenate([np.array([V0], np.int64),
                             inp["dec_tokens"].astype(np.int64)])
    enc_emb = np.asarray(inp["enc_emb"], np.float32)
    dec_emb = np.asarray(inp["dec_emb"], np.float32)

    ctx = np.ascontiguousarray(enc_emb[idx_enc])                       # [320, 1024]
    decx = np.ascontiguousarray(dec_emb[tokens])                       # [65, 1024]
    we = np.ascontiguousarray(np.asarray(inp["attn_w"], np.float32)[0, E:]).reshape(1, E)

    w_ih = np.asarray(inp["gru_w_ih"], np.float32)                     # [3072, 2048]
    w_hh = np.asarray(inp["gru_w_hh"], np.float32)                     # [3072, 1024]
    b_ih = np.asarray(inp["gru_b_ih"], np.float32)
    b_hh = np.asarray(inp["gru_b_hh"], np.float32)
    assert not np.any(b_hh[2 * E:]), "nonzero b_hh n-gate not supported on device"

    whh_arr = _arrange_w(w_hh).astype(NP_BF16)                         # [128, 24576]
    wc_arr = _arrange_w(np.ascontiguousarray(w_ih[:, :E])).astype(NP_BF16)
    wx_arr = _arrange_w(np.ascontiguousarray(w_ih[:, E:])).astype(NP_BF16)
    bias = b_ih.copy()
    bias[:2 * E] += b_hh[:2 * E]
    bias_arr = _bias_tall(bias)                                        # [128, 768] f32

    out_w = np.asarray(inp["out_w"], np.float32)
    out_b = np.asarray(inp["out_b"], np.float32)
    owp = np.zeros((NCORES * VP, E), np.float32)
    owp[:V] = out_w
    obp = np.zeros(NCORES * VP, np.float32)
    obp[:V] = out_b

    base = {"ctx": ctx, "decx": decx, "we": we, "whh": whh_arr,
            "wc": wc_arr, "wx": wx_arr, "bias": bias_arr}
    in_maps = []
    for i in range(NCORES):
        s = owp[i * VP:(i + 1) * VP]                                   # [4096, 1024]
        owt = np.ascontiguousarray(
            s.reshape(VP, EC, 128).transpose(2, 1, 0)).astype(NP_BF16).reshape(128, EC * VP)
        m = dict(base)
        m["owt"] = owt
        m["outb"] = np.ascontiguousarray(obp[i * VP:(i + 1) * VP]).reshape(1, VP)
        in_maps.append(m)
    return in_maps


def kernel(**inputs):
    in_maps = _prep_inputs({k: np.asarray(v) for k, v in inputs.items()})
    wb = any(bool(np.any(m["outb"])) for m in in_maps)
    key = ("nc", wb)
    if key not in _CACHE:
        _CACHE[key] = build_program(with_bias=wb)
    _CACHE["nc"] = _CACHE[key]
    nc = _CACHE[key]
    res = run_bass_kernel_spmd(nc, in_maps, core_ids=list(range(NCORES)))
    slices = [res.results[i]["out"] for i in range(NCORES)]            # each [65, 4096]
    logits = np.concatenate(slices, axis=1)[:, :V]
    return np.ascontiguousarray(logits.astype(np.float32))


# revision 6
# speedup vs baseline: 1.1742x; 1.1742x over previous
"""Trainium2 Bass kernel for nn_LEAP_74371653697613 (GRU decoder w/ additive attention).

v2: Picard-sweep formulation of the recurrence.

Structure exploited:
  - softmax(ctx_score + h.w_h) == softmax(ctx_score): attention weights constant
    across steps -> context c computed once.
  - gi_t = W_ih @ [c; x_t] + b_ih precomputed batched (teacher forcing).
  - The h-recurrence h_t = (1-z_t) n_t + z_t h_{t-1} is solved by fixed-point
    iteration over the WHOLE sequence: each sweep computes gh = W_hh @ H_prev as
    ONE batched [65,3072] matmul (W_hh streamed once per sweep instead of once
    per step), gates batched, then the exact sequential structure is restored by
    the DVE prefix-scan  state = (z_t * state) - (z_t - 1)*n_t  per hidden chunk.
    Contraction ~0.3x/sweep; KS sweeps reach the bf16 noise floor (~2.6e-3 rel).
  - Sweep 1 has H_prev === h0, so its gh is a single col-tiled matvec W_hh @ h0
    broadcast across t (cheap) instead of the batched matmul.
  - logits = relu(H) @ out_w^T batched at the end, vocab-sharded 8 ways
    (each core owns a 4096-row slice of out_w); out_w prefetched during sweeps.

outer_reps repeats phases 2-5 inside one NEFF for slope-based timing; the
shipped kernel uses outer_reps=1.
"""
import os
import sys
import numpy as np

for _p in ("/opt/trn_rl_repo", "/root/.axon_site/_ro/trn_rl_repo"):
    if os.path.isdir(_p) and _p not in sys.path:
        sys.path.insert(0, _p)

import concourse.bass as bass
import concourse.bacc as bacc
import concourse.tile as tile
import concourse.mybir as mybir
from concourse.bass_utils import run_bass_kernel_spmd
from concourse.masks import make_identity

F32 = mybir.dt.float32
BF16 = mybir.dt.bfloat16
AF = mybir.ActivationFunctionType
ALU = mybir.AluOpType
NP_BF16 = mybir.dt.np(BF16)

E = 1024          # emb dim
EC = 8            # E / 128 chunks
T = 65            # decode steps (1 SOS + 64)
L = 320           # context rows (128 + 64 + 128)
V0 = 32000
V = V0 + 2        # 32002
NCORES = 8
VP = 4096         # per-core padded vocab slice (8 * 4096 = 32768 >= 32002)
G = 4             # gate regions (each 256 hidden units x 3 gates)
RG = 768          # region width (3 gates x 256)
CW = G * RG       # per-K-chunk weight width (3072)
KS = 6            # Picard sweeps (incl. the specialized first sweep)
TP = 66           # padded t-stride in hprevT: slot 0 = h0, slots 1..65 = scan out

_CACHE = {}


def _arrange_w(w):
    """[3072, 1024] -> [128, 8*4*768]: out[p, ((c*4)+j)*768 + g*256+mm]
    = w[g*1024 + j*256 + mm, c*128 + p]."""
    x = w.reshape(3, 4, 256, EC, 128)            # g, j, mm, c, p
    x = np.transpose(x, (4, 3, 1, 0, 2))         # p, c, j, g, mm
    return np.ascontiguousarray(x).reshape(128, EC * CW)


def _bias_tall(b_rzn):
    x = b_rzn.reshape(3, 4, 256)                 # g, j, mm
    x = np.transpose(x, (1, 0, 2)).reshape(4, RG)  # j, (g mm)
    out = np.zeros((128, RG), np.float32)
    out[::32, :] = x
    return out


def build_program(ksweeps=KS, do_final=True, outer_reps=1, with_bias=True):
    nc = bacc.Bacc("TRN2", target_bir_lowering=False, debug=False, num_devices=NCORES)

    ctx_d = nc.dram_tensor("ctx", [L, E], F32, kind="ExternalInput").ap()
    decx_d = nc.dram_tensor("decx", [T, E], F32, kind="ExternalInput").ap()
    we_d = nc.dram_tensor("we", [1, E], F32, kind="ExternalInput").ap()
    whh_d = nc.dram_tensor("whh", [128, EC * CW], BF16, kind="ExternalInput").ap()
    wc_d = nc.dram_tensor("wc", [128, EC * CW], BF16, kind="ExternalInput").ap()
    wx_d = nc.dram_tensor("wx", [128, EC * CW], BF16, kind="ExternalInput").ap()
    bias_d = nc.dram_tensor("bias", [128, RG], F32, kind="ExternalInput").ap()
    owt_d = nc.dram_tensor("owt", [128, EC * VP], BF16, kind="ExternalInput").ap()
    outb_d = nc.dram_tensor("outb", [1, VP], F32, kind="ExternalInput").ap()
    out_d = nc.dram_tensor("out", [T, VP], BF16, kind="ExternalOutput").ap()

    with tile.TileContext(nc) as tc:
        with tc.tile_pool(name="persist", bufs=1) as pp:
            # ---------- persistent constants ----------
            whh = pp.tile([128, EC * CW], BF16)
            for c in range(EC):
                nc.sync.dma_start(whh[:, c * CW:(c + 1) * CW],
                                  whh_d[:, c * CW:(c + 1) * CW])

            ident = pp.tile([128, 128], F32)
            make_identity(nc, ident[:])
            ident_bf = pp.tile([128, 128], BF16)
            nc.vector.tensor_copy(ident_bf[:], ident[:])

            ones_tall = pp.tile([128, T], F32)
            nc.gpsimd.memset(ones_tall[:], 1.0)

            bias_tall = pp.tile([128, RG], F32)
            nc.sync.dma_start(bias_tall[:], bias_d[:])

            girz = pp.tile([T, G, 512], BF16)      # gi rz-part, partition = t
            gin65 = pp.tile([T, 1024], F32)        # gi n-part, partition = t
            gic_tall = pp.tile([128, RG], F32)     # const part of gi, rows at 32j
            h_stat = pp.tile([128, EC], F32)       # h0 chunks (scan initial)
            h0bf = pp.tile([128, EC], BF16)
            gh0 = pp.tile([128, RG], F32)          # W_hh @ h0, region rows at 32j
            hprevT = pp.tile([128, EC * TP], BF16)  # slot (c,0)=h0; (c,1..65)=h_1..h_65
            htf = pp.tile([128, EC * T], BF16)      # relu(h_1..h_65)
            cT_bf = pp.tile([128, EC], BF16)
            dxT_bf = pp.tile([128, EC, T], BF16)

            # h0 = dec_emb[SOS] = decx row 0 in stationary layout
            nc.sync.dma_start(h_stat[:], decx_d[0:1, :].rearrange("o (c p) -> (o p) c", p=128))
            nc.vector.tensor_copy(h0bf[:], h_stat[:])

            whhv = whh[:].rearrange("p (c j m) -> p c j m", c=EC, j=G)
            hprevT_v = hprevT[:].rearrange("p (c t) -> p c t", c=EC)

            # ---------- phase 1: attention (constant across steps) ----------
            with tc.tile_pool(name="ph1", bufs=1) as p1, \
                 tc.tile_pool(name="ph1ps", bufs=1, space="PSUM") as p1ps:
                ones_col = p1.tile([128, 1], F32)
                nc.gpsimd.memset(ones_col[:], 1.0)
                ones_row = p1.tile([1, 128], F32)
                nc.gpsimd.memset(ones_row[:], 1.0)
                we_sb = p1.tile([1, E], F32)
                nc.sync.dma_start(we_sb[:], we_d[:])
                rows3 = (128, 128, 64)
                ctx_sb = []
                for i, rows in enumerate(rows3):
                    t_ = p1.tile([128, E], F32, tag=f"ctx{i}")
                    nc.sync.dma_start(t_[:rows, :], ctx_d[128 * i:128 * i + rows, :])
                    ctx_sb.append(t_)
                decx_sb = p1.tile([T, E], F32)
                nc.sync.dma_start(decx_sb[:], decx_d[:])

                werep_ps = p1ps.tile([128, E], F32, space="PSUM")
                for half in range(2):
                    nc.tensor.matmul(werep_ps[:, 512 * half:512 * (half + 1)],
                                     lhsT=ones_row[:1, :],
                                     rhs=we_sb[:1, 512 * half:512 * (half + 1)],
                                     start=True, stop=True)
                werep = p1.tile([128, E], F32)
                nc.vector.tensor_copy(werep[:], werep_ps[:])

                scratch = p1.tile([128, E], F32)
                escore = [p1.tile([128, 1], F32, tag=f"esc{i}", name=f"esc{i}")
                          for i in range(3)]
                for i, rows in enumerate(rows3):
                    sc = p1.tile([128, 1], F32, tag=f"sc{i}")
                    nc.vector.tensor_tensor(out=scratch[:rows, :],
                                            in0=ctx_sb[i][:rows, :],
                                            in1=werep[:rows, :], op=ALU.mult)
                    nc.vector.tensor_reduce(out=sc[:rows, :], in_=scratch[:rows, :],
                                            axis=mybir.AxisListType.X, op=ALU.add)
                    nc.scalar.activation(escore[i][:rows, :], sc[:rows, :], AF.Exp)
                ssum_ps = p1ps.tile([1, 1], F32, space="PSUM")
                for i, rows in enumerate(rows3):
                    nc.tensor.matmul(ssum_ps[:1, :1], lhsT=escore[i][:rows, :1],
                                     rhs=ones_col[:rows, :1],
                                     start=(i == 0), stop=(i == 2))
                rsum = p1.tile([1, 1], F32)
                nc.vector.reciprocal(rsum[:], ssum_ps[:1, :1])

                cun_ps = p1ps.tile([1, E], F32, space="PSUM")
                for half in range(2):
                    for i, rows in enumerate(rows3):
                        nc.tensor.matmul(cun_ps[:1, 512 * half:512 * (half + 1)],
                                         lhsT=escore[i][:rows, :1],
                                         rhs=ctx_sb[i][:rows, 512 * half:512 * (half + 1)],
                                         start=(i == 0), stop=(i == 2))
                c_sb = p1.tile([1, E], F32)
                nc.vector.tensor_scalar_mul(c_sb[:], cun_ps[:1, :], rsum[:1, :1])

                cT_ps = p1ps.tile([128, EC], F32, space="PSUM")
                for k in range(EC):
                    nc.tensor.transpose(out=cT_ps[:, k:k + 1],
                                        in_=c_sb[:1, 128 * k:128 * (k + 1)],
                                        identity=ident[:1, :1])
                nc.vector.tensor_copy(cT_bf[:], cT_ps[:])

                dxT_ps = p1ps.tile([128, T], F32, space="PSUM")
                for k in range(EC):
                    nc.tensor.transpose(out=dxT_ps[:, :],
                                        in_=decx_sb[:T, 128 * k:128 * (k + 1)],
                                        identity=ident[:T, :T])
                    nc.vector.tensor_copy(dxT_bf[:, k, :], dxT_ps[:, :])

            for rep in range(outer_reps):
                # ---------- phase 2: gic = W_ih[:, :E] @ c + biases ----------
                with tc.tile_pool(name=f"pwc{rep}", bufs=1) as pwc, \
                     tc.tile_pool(name=f"pwcps{rep}", bufs=1, space="PSUM") as pwcps:
                    wc_sb = pwc.tile([128, EC * CW], BF16)
                    for c in range(EC):
                        nc.sync.dma_start(wc_sb[:, c * CW:(c + 1) * CW],
                                          wc_d[:, c * CW:(c + 1) * CW])
                    wcv = wc_sb[:].rearrange("p (c j m) -> p c j m", c=EC, j=G)
                    gic_ps = pwcps.tile([128, 1024], F32, space="PSUM")
                    for c in range(EC):
                        for j in range(G):
                            nc.tensor.matmul(gic_ps[32 * j:32 * j + 1, 0:512],
                                             lhsT=cT_bf[:, c:c + 1],
                                             rhs=wcv[:, c, j, 0:512],
                                             start=(c == 0), stop=False,
                                             tile_position=(0, 32 * j))
                            nc.tensor.matmul(gic_ps[32 * j:32 * j + 1, 512:768],
                                             lhsT=cT_bf[:, c:c + 1],
                                             rhs=wcv[:, c, j, 512:768],
                                             start=(c == 0), stop=False,
                                             tile_position=(0, 32 * j))
                    for j in range(G):
                        nc.tensor.matmul(gic_ps[32 * j:32 * j + 1, 0:512],
                                         lhsT=ones_tall[32 * j:32 * j + 1, 0:1],
                                         rhs=bias_tall[32 * j:32 * j + 1, 0:512],
                                         start=False, stop=True,
                                         tile_position=(32 * j, 32 * j))
                        nc.tensor.matmul(gic_ps[32 * j:32 * j + 1, 512:768],
                                         lhsT=ones_tall[32 * j:32 * j + 1, 0:1],
                                         rhs=bias_tall[32 * j:32 * j + 1, 512:768],
                                         start=False, stop=True,
                                         tile_position=(32 * j, 32 * j))
                    for j in range(G):
                        if j % 2 == 0:
                            nc.scalar.copy(gic_tall[32 * j:32 * j + 1, :],
                                           gic_ps[32 * j:32 * j + 1, 0:RG])
                        else:
                            nc.vector.tensor_copy(gic_tall[32 * j:32 * j + 1, :],
                                                  gic_ps[32 * j:32 * j + 1, 0:RG])

                # ---------- phase 3: gi[t] = gic + W_ih[:, E:] @ x_t (batched) ----------
                with tc.tile_pool(name=f"pwx{rep}", bufs=1) as pwx, \
                     tc.tile_pool(name=f"pwxps{rep}", bufs=1, space="PSUM") as pwxps:
                    wx_sb = pwx.tile([128, EC * CW], BF16)
                    for c in range(EC):
                        nc.sync.dma_start(wx_sb[:, c * CW:(c + 1) * CW],
                                          wx_d[:, c * CW:(c + 1) * CW])
                    wxv = wx_sb[:].rearrange("p (c j m) -> p c j m", c=EC, j=G)
                    rzts = [pwxps.tile([T, 512], F32, space="PSUM", tag=f"grz{j}",
                                       name=f"grz{j}") for j in range(G)]
                    npts = [pwxps.tile([T, 256], F32, space="PSUM", tag=f"gn{j}",
                                       name=f"gn{j}") for j in range(G)]
                    for c in range(EC):
                        for j in range(G):
                            nc.tensor.matmul(rzts[j][:T, :], lhsT=dxT_bf[:, c, :],
                                             rhs=wxv[:, c, j, 0:512],
                                             start=(c == 0), stop=False)
                            nc.tensor.matmul(npts[j][:T, :], lhsT=dxT_bf[:, c, :],
                                             rhs=wxv[:, c, j, 512:768],
                                             start=(c == 0), stop=False)
                    for j in range(G):
                        nc.tensor.matmul(rzts[j][:T, :],
                                         lhsT=ones_tall[32 * j:32 * j + 1, :T],
                                         rhs=gic_tall[32 * j:32 * j + 1, 0:512],
                                         start=False, stop=True,
                                         tile_position=(32 * j, 0))
                        nc.vector.tensor_copy(girz[:, j, :], rzts[j][:T, :])
                        nc.tensor.matmul(npts[j][:T, :],
                                         lhsT=ones_tall[32 * j:32 * j + 1, :T],
                                         rhs=gic_tall[32 * j:32 * j + 1, 512:768],
                                         start=False, stop=True,
                                         tile_position=(32 * j, 0))
                        nc.vector.tensor_copy(gin65[:, 256 * j:256 * (j + 1)],
                                              npts[j][:T, :])

                # prefetch final-phase weights (after whh/wc/wx in program order)
                if rep == 0:
                    owt_sb = pp.tile([128, EC * VP], BF16)
                    outb_sb = pp.tile([1, VP], F32)
                    if do_final:
                        nc.sync.dma_start(owt_sb[:], owt_d[:])
                        nc.sync.dma_start(outb_sb[:], outb_d[:])

                # ---------- phase 4: Picard sweeps ----------
                # hprevT[:, c, 0] = h0 (sweeps 2+ read it; cols 1: from scans)
                nc.vector.tensor_copy(hprevT_v[:, :, 0:1], h_stat[:].unsqueeze(2))

                with tc.tile_pool(name=f"sw{rep}", bufs=4) as psw, \
                     tc.tile_pool(name=f"swg{rep}", bufs=2) as psg2, \
                     tc.tile_pool(name=f"swps{rep}", bufs=2, space="PSUM") as pps1, \
                     tc.tile_pool(name=f"swpsT{rep}", bufs=1, space="PSUM") as pps2:
                    # sweep-1 prologue: gh0 = W_hh @ h0 (col-tiled matvec, M=1).
                    # psg0 borrows the zT buffer (tag reuse; lifetimes disjoint).
                    psg0 = pps2.tile([128, 1024], F32, space="PSUM", tag="zT")
                    for c in range(EC):
                        for j in range(G):
                            nc.tensor.matmul(psg0[32 * j:32 * j + 1, 0:512],
                                             lhsT=h0bf[:, c:c + 1],
                                             rhs=whhv[:, c, j, 0:512],
                                             start=(c == 0), stop=(c == EC - 1),
                                             tile_position=(0, 32 * j))
                            nc.tensor.matmul(psg0[32 * j:32 * j + 1, 512:768],
                                             lhsT=h0bf[:, c:c + 1],
                                             rhs=whhv[:, c, j, 512:768],
                                             start=(c == 0), stop=(c == EC - 1),
                                             tile_position=(0, 32 * j))
                    for j in range(G):
                        if j % 2 == 0:
                            nc.scalar.copy(gh0[32 * j:32 * j + 1, :],
                                           psg0[32 * j:32 * j + 1, 0:RG])
                        else:
                            nc.vector.tensor_copy(gh0[32 * j:32 * j + 1, :],
                                                  psg0[32 * j:32 * j + 1, 0:RG])

                    for k in range(ksweeps):
                        first = (k == 0)
                        sgs, npres = [], []
                        # rz pass: 4 regions, gi fold + 8 K-chunks each
                        for j in range(G):
                            rz = pps1.tile([T, 512], F32, space="PSUM", tag="rz")
                            nc.tensor.matmul(rz[:T, :], lhsT=ident_bf[:T, :T],
                                             rhs=girz[:, j, :], start=True, stop=False)
                            if first:
                                nc.tensor.matmul(rz[:T, :],
                                                 lhsT=ones_tall[32 * j:32 * j + 1, :T],
                                                 rhs=gh0[32 * j:32 * j + 1, 0:512],
                                                 start=False, stop=True,
                                                 tile_position=(32 * j, 0))
                            else:
                                for c in range(EC):
                                    nc.tensor.matmul(rz[:T, :],
                                                     lhsT=hprevT[:, c * TP:c * TP + T],
                                                     rhs=whhv[:, c, j, 0:512],
                                                     start=False, stop=(c == EC - 1))
                            sg = psw.tile([T, 512], F32, tag="sg")
                            nc.scalar.activation(sg[:], rz[:T, :], AF.Sigmoid)
                            sgs.append(sg)
                        # n pass
                        for j in range(G):
                            nps_j = pps1.tile([T, 256], F32, space="PSUM", tag="n")
                            if first:
                                nc.tensor.matmul(nps_j[:T, :],
                                                 lhsT=ones_tall[32 * j:32 * j + 1, :T],
                                                 rhs=gh0[32 * j:32 * j + 1, 512:768],
                                                 start=True, stop=True,
                                                 tile_position=(32 * j, 0))
                            else:
                                for c in range(EC):
                                    nc.tensor.matmul(nps_j[:T, :],
                                                     lhsT=hprevT[:, c * TP:c * TP + T],
                                                     rhs=whhv[:, c, j, 512:768],
                                                     start=(c == 0), stop=(c == EC - 1))
                            t1 = psg2.tile([T, 256], F32, tag="t1")
                            nc.vector.tensor_tensor(out=t1[:], in0=sgs[j][:, 0:256],
                                                    in1=nps_j[:T, :], op=ALU.mult)
                            npre = psw.tile([T, 256], F32, tag="npre")
                            nc.vector.tensor_tensor(out=npre[:], in0=t1[:],
                                                    in1=gin65[:, 256 * j:256 * (j + 1)],
                                                    op=ALU.add)
                            npres.append(npre)
                        # transposes into [128, (half, cc, t)] PSUM; chunk cc at
                        # column 512*(cc//4) + 65*(cc%4)
                        zT = pps2.tile([128, 1024], F32, space="PSUM", tag="zT")
                        npT = pps2.tile([128, 1024], F32, space="PSUM", tag="npT")
                        for cc in range(EC):
                            j, k2 = cc // 2, cc % 2
                            col = 512 * (cc // 4) + T * (cc % 4)
                            nc.tensor.transpose(out=zT[:, col:col + T],
                                                in_=sgs[j][:T, 256 + 128 * k2:256 + 128 * (k2 + 1)],
                                                identity=ident[:T, :T])
                            nc.tensor.transpose(out=npT[:, col:col + T],
                                                in_=npres[j][:T, 128 * k2:128 * (k2 + 1)],
                                                identity=ident[:T, :T])
                        nT = psg2.tile([128, 1024], F32, tag="nT")
                        wsb = psg2.tile([128, 1024], F32, tag="wsb")
                        for h in range(2):
                            s = slice(512 * h, 512 * h + 4 * T)
                            nc.scalar.activation(nT[:, s], npT[:, s], AF.Tanh)
                            # (z - 1) * n
                            nc.vector.scalar_tensor_tensor(out=wsb[:, s], in0=zT[:, s],
                                                           scalar=1.0, in1=nT[:, s],
                                                           op0=ALU.subtract, op1=ALU.mult)
                        for cc in range(EC):
                            col = 512 * (cc // 4) + T * (cc % 4)
                            # state = z*state - (z-1)*n; writes h_1..h_65 into
                            # slots (cc, 1..65); next sweep's lhsT reads (cc, 0..64)
                            nc.vector.tensor_tensor_scan(
                                out=hprevT[:, cc * TP + 1:cc * TP + 1 + T],
                                data0=zT[:, col:col + T], data1=wsb[:, col:col + T],
                                initial=h_stat[:, cc:cc + 1],
                                op0=ALU.mult, op1=ALU.subtract)

                # ---------- phase 5: logits = relu(H) @ out_w^T + out_b ----------
                nc.scalar.activation(htf[:].rearrange("p (c t) -> p c t", c=EC),
                                     hprevT_v[:, :, 1:TP], AF.Relu)
                owtv = owt_sb[:].rearrange("p (c v) -> p c v", c=EC)
                htv = htf[:].rearrange("p (c t) -> p c t", c=EC)
                if not do_final and rep == 0:
                    nc.sync.dma_start(out_d[0:T, 0:T], htf[:T, 0:T])
                NVB = VP // 512
                with tc.tile_pool(name=f"fin{rep}", bufs=4) as pf, \
                     tc.tile_pool(name=f"finps{rep}", bufs=1, space="PSUM") as pfps:
                    if do_final:
                        # c-outer: load each stationary h-chunk once, stream all
                        # 8 vocab blocks from it (8 concurrent PSUM groups)
                        opss = [pfps.tile([T, 512], F32, space="PSUM",
                                          tag=f"ops{vb}", name=f"ops{vb}")
                                for vb in range(NVB)]
                        for c in range(EC):
                            for vb in range(NVB):
                                nc.tensor.matmul(opss[vb][:T, :], lhsT=htv[:, c, :],
                                                 rhs=owtv[:, c, 512 * vb:512 * (vb + 1)],
                                                 start=(c == 0),
                                                 stop=(not with_bias and c == EC - 1))
                        for vb in range(NVB):
                            if with_bias:
                                nc.tensor.matmul(opss[vb][:T, :], lhsT=ones_tall[:1, :T],
                                                 rhs=outb_sb[:1, 512 * vb:512 * (vb + 1)],
                                                 start=False, stop=True)
                            osb = pf.tile([T, 512], BF16, tag="osb")
                            if vb % 2 == 0:
                                nc.vector.tensor_copy(osb[:], opss[vb][:T, :])
                            else:
                                nc.scalar.copy(osb[:], opss[vb][:T, :])
                            nc.sync.dma_start(out_d[:, 512 * vb:512 * (vb + 1)], osb[:])

    nc.compile()
    return nc


def _prep_inputs(inp):
    idx_enc = np.concatenate([inp["input_diagnosis"], inp["input_procedure"],
                              inp["input_medicine"]]).astype(np.int64)
    tokens = np.concatenate([np.array([V0], np.int64),
                             inp["dec_tokens"].astype(np.int64)])
    enc_emb = np.asarray(inp["enc_emb"], np.float32)
    dec_emb = np.asarray(inp["dec_emb"], np.float32)

    ctx = np.ascontiguousarray(enc_emb[idx_enc])                       # [320, 1024]
    decx = np.ascontiguousarray(dec_emb[tokens])                       # [65, 1024]
    we = np.ascontiguousarray(np.asarray(inp["attn_w"], np.float32)[0, E:]).reshape(1, E)

    w_ih = np.asarray(inp["gru_w_ih"], np.float32)                     # [3072, 2048]
    w_hh = np.asarray(inp["gru_w_hh"], np.float32)                     # [3072, 1024]
    b_ih = np.asarray(inp["gru_b_ih"], np.float32)
    b_hh = np.asarray(inp["gru_b_hh"], np.float32)
    assert not np.any(b_hh[2 * E:]), "nonzero b_hh n-gate not supported on device"

    whh_arr = _arrange_w(w_hh).astype(NP_BF16)                         # [128, 24576]
    wc_arr = _arrange_w(np.ascontiguousarray(w_ih[:, :E])).astype(NP_BF16)
    wx_arr = _arrange_w(np.ascontiguousarray(w_ih[:, E:])).astype(NP_BF16)
    bias = b_ih.copy()
    bias[:2 * E] += b_hh[:2 * E]
    bias_arr = _bias_tall(bias)                                        # [128, 768] f32

    out_w = np.asarray(inp["out_w"], np.float32)
    out_b = np.asarray(inp["out_b"], np.float32)
    owp = np.zeros((NCORES * VP, E), np.float32)
    owp[:V] = out_w
    obp = np.zeros(NCORES * VP, np.float32)
    obp[:V] = out_b

    base = {"ctx": ctx, "decx": decx, "we": we, "whh": whh_arr,
            "wc": wc_arr, "wx": wx_arr, "bias": bias_arr}
    in_maps = []
    for i in range(NCORES):
        s = owp[i * VP:(i + 1) * VP]                                   # [4096, 1024]
        owt = np.ascontiguousarray(
            s.reshape(VP, EC, 128).transpose(2, 1, 0)).astype(NP_BF16).reshape(128, EC * VP)
        m = dict(base)
        m["owt"] = owt
        m["outb"] = np.ascontiguousarray(obp[i * VP:(i + 1) * VP]).reshape(1, VP)
        in_maps.append(m)
    return in_maps


def kernel(**inputs):
    in_maps = _prep_inputs({k: np.asarray(v) for k, v in inputs.items()})
    wb = any(bool(np.any(m["outb"])) for m in in_maps)
    key = ("nc", wb)
    if key not in _CACHE:
        _CACHE[key] = build_program(with_bias=wb)
    _CACHE["nc"] = _CACHE[key]
    nc = _CACHE[key]
    res = run_bass_kernel_spmd(nc, in_maps, core_ids=list(range(NCORES)))
    slices = [res.results[i]["out"] for i in range(NCORES)]            # each [65, 4096]
    logits = np.concatenate(slices, axis=1)[:, :V]
    return np.ascontiguousarray(logits.astype(np.float32))
